# revision 1
# baseline (speedup 1.0000x reference)
"""MoE linear (modality-routed) Trainium2 kernel.

out[n] = x[n] @ W[modality_ids[n]].T + b[modality_ids[n]]

Strategy (data parallel over 8 cores, weight replicated):
- Host: per core shard of 16384 tokens, stable-argsort tokens by expert.
  Groups padded to a shared per-expert capacity (multiple of 128) so one
  SPMD NEFF serves all cores; per-tile expert is a compile-time constant.
- Device per 128-token tile: indirect-DMA gather of x rows -> PE transpose
  (contraction dim to partitions) -> 4 accumulating fp32r matmuls against
  SBUF-resident W^T -> bias add on DVE -> indirect-DMA scatter to the
  token's original row. Padding slots scatter to an out-of-bounds index
  and are dropped via bounds_check.
"""

import sys

if "/opt/trn_rl_repo" not in sys.path:
    sys.path.insert(0, "/opt/trn_rl_repo")

import numpy as np

import concourse.bass as bass  # noqa: F401
import concourse.tile as tile
from concourse import bacc, mybir
from concourse.bass import IndirectOffsetOnAxis
from concourse.bass_utils import run_bass_kernel_spmd
from concourse.masks import make_identity

N_CORES = 8
N_TOKENS = 131072
N_SHARD = N_TOKENS // N_CORES  # 16384
D_IN = 512
D_OUT = 512
N_EXPERTS = 3
P = 128
KC = D_IN // P  # 4 contraction chunks

_NC_CACHE = {}


def build_nc(n_shard, caps, num_devices=N_CORES):
    """Build + compile the SPMD Bass kernel for given per-expert capacities."""
    key = (n_shard, tuple(caps), num_devices)
    if key in _NC_CACHE:
        return _NC_CACHE[key]
    nt = sum(caps) // P
    experts_of_tile = []
    for e, c in enumerate(caps):
        experts_of_tile += [e] * (c // P)

    nc = bacc.Bacc(
        "TRN2", target_bir_lowering=False, debug=False, num_devices=num_devices
    )
    f32 = mybir.dt.float32
    f32r = mybir.dt.float32r
    i32 = mybir.dt.int32

    x = nc.dram_tensor("x", [n_shard, D_IN], f32, kind="ExternalInput").ap()
    wt = nc.dram_tensor(
        "wt", [D_IN, N_EXPERTS * D_OUT], f32r, kind="ExternalInput"
    ).ap()
    bb = nc.dram_tensor(
        "bias_bc", [P, N_EXPERTS * D_OUT], f32, kind="ExternalInput"
    ).ap()
    gsrc = nc.dram_tensor("gsrc", [P, nt], i32, kind="ExternalInput").ap()
    gdst = nc.dram_tensor("gdst", [P, nt], i32, kind="ExternalInput").ap()
    y = nc.dram_tensor("y", [n_shard, D_OUT], f32, kind="ExternalOutput").ap()

    with tile.TileContext(nc) as tc:
        with (
            tc.tile_pool(name="const", bufs=1) as cpool,
            tc.tile_pool(name="xg", bufs=6) as xg_pool,
            tc.tile_pool(name="xt", bufs=4) as xt_pool,
            tc.tile_pool(name="outp", bufs=6) as out_pool,
            tc.tile_pool(name="ptr", bufs=3, space="PSUM") as ptr_pool,
            tc.tile_pool(name="pmm", bufs=3, space="PSUM") as pmm_pool,
        ):
            ident = cpool.tile([P, P], f32)
            make_identity(nc, ident[:])

            # W^T resident in SBUF: block (e, kc) is [k=128, o=512]
            w_sb = cpool.tile([P, N_EXPERTS * KC * D_OUT], f32r)
            for e in range(N_EXPERTS):
                for kc in range(KC):
                    nc.sync.dma_start(
                        out=w_sb[:, (e * KC + kc) * D_OUT : (e * KC + kc + 1) * D_OUT],
                        in_=wt[kc * P : (kc + 1) * P, e * D_OUT : (e + 1) * D_OUT],
                    )
            bias_sb = cpool.tile([P, N_EXPERTS * D_OUT], f32)
            nc.sync.dma_start(out=bias_sb[:], in_=bb[:])
            gsrc_sb = cpool.tile([P, nt], i32)
            nc.sync.dma_start(out=gsrc_sb[:], in_=gsrc[:])
            gdst_sb = cpool.tile([P, nt], i32)
            nc.sync.dma_start(out=gdst_sb[:], in_=gdst[:])

            for t in range(nt):
                e = experts_of_tile[t]
                xg = xg_pool.tile([P, D_IN], f32)
                nc.gpsimd.indirect_dma_start(
                    out=xg[:],
                    out_offset=None,
                    in_=x[:],
                    in_offset=IndirectOffsetOnAxis(ap=gsrc_sb[:, t : t + 1], axis=0),
                )
                ptr = ptr_pool.tile([P, D_IN], f32)
                for kc in range(KC):
                    nc.tensor.transpose(
                        ptr[:, kc * P : (kc + 1) * P],
                        xg[:, kc * P : (kc + 1) * P],
                        ident[:],
                    )
                xt = xt_pool.tile([P, D_IN], f32r)
                nc.vector.tensor_copy(xt[:], ptr[:])
                pmm = pmm_pool.tile([P, D_OUT], f32)
                for kc in range(KC):
                    nc.tensor.matmul(
                        pmm[:],
                        lhsT=xt[:, kc * P : (kc + 1) * P],
                        rhs=w_sb[
                            :, (e * KC + kc) * D_OUT : (e * KC + kc + 1) * D_OUT
                        ],
                        start=(kc == 0),
                        stop=(kc == KC - 1),
                    )
                osb = out_pool.tile([P, D_OUT], f32)
                nc.vector.tensor_add(
                    out=osb[:],
                    in0=pmm[:],
                    in1=bias_sb[:, e * D_OUT : (e + 1) * D_OUT],
                )
                nc.gpsimd.indirect_dma_start(
                    out=y[:],
                    out_offset=IndirectOffsetOnAxis(ap=gdst_sb[:, t : t + 1], axis=0),
                    in_=osb[:],
                    in_offset=None,
                    bounds_check=n_shard - 1,
                    oob_is_err=False,
                )

    nc.compile()
    _NC_CACHE[key] = nc
    return nc


def make_routing(ids_shard, caps):
    """gsrc/gdst [P, nt] int32 for one core. Padding: src->0, dst->n_shard (OOB)."""
    n_shard = ids_shard.shape[0]
    npad = sum(caps)
    nt = npad // P
    order = np.argsort(ids_shard, kind="stable").astype(np.int32)
    cnt = np.bincount(ids_shard, minlength=N_EXPERTS)
    gs = np.zeros(npad, np.int32)
    gd = np.full(npad, n_shard, np.int32)
    base = 0
    off = 0
    for e in range(N_EXPERTS):
        c = int(cnt[e])
        seg = order[off : off + c]
        gs[base : base + c] = seg
        gd[base : base + c] = seg
        base += caps[e]
        off += c
    gsrc = np.ascontiguousarray(gs.reshape(nt, P).T)
    gdst = np.ascontiguousarray(gd.reshape(nt, P).T)
    return gsrc, gdst


def prepare(inputs):
    """Shared host-side prep: returns (nc, in_maps)."""
    x = np.ascontiguousarray(np.asarray(inputs["x"], dtype=np.float32))
    ids = np.asarray(inputs["modality_ids"]).astype(np.int64)
    weight = np.asarray(inputs["weight"], dtype=np.float32)
    b = np.asarray(inputs["bias"], dtype=np.float32)

    wt = np.ascontiguousarray(weight.T)  # [D_IN, E*D_OUT]
    bias_bc = np.ascontiguousarray(
        np.broadcast_to(b[None, :], (P, N_EXPERTS * D_OUT))
    )

    counts = np.stack(
        [
            np.bincount(ids[c * N_SHARD : (c + 1) * N_SHARD], minlength=N_EXPERTS)
            for c in range(N_CORES)
        ]
    )
    caps = [int(-(-counts[:, e].max() // P) * P) for e in range(N_EXPERTS)]

    nc = build_nc(N_SHARD, caps)
    in_maps = []
    for c in range(N_CORES):
        ids_c = ids[c * N_SHARD : (c + 1) * N_SHARD]
        gsrc, gdst = make_routing(ids_c, caps)
        in_maps.append(
            {
                "x": np.ascontiguousarray(x[c * N_SHARD : (c + 1) * N_SHARD]),
                "wt": wt,
                "bias_bc": bias_bc,
                "gsrc": gsrc,
                "gdst": gdst,
            }
        )
    return nc, in_maps


def run(inputs, trace=False):
    """Returns (out, BassKernelResults)."""
    nc, in_maps = prepare(inputs)
    res = run_bass_kernel_spmd(nc, in_maps, list(range(N_CORES)), trace=trace)
    out = np.concatenate(
        [res.results[c]["y"] for c in range(N_CORES)], axis=0
    ).astype(np.float32)
    return out, res


def kernel(**inputs):
    out, _ = run(inputs, trace=False)
    return out



# revision 3
# speedup vs baseline: 96.1419x; 96.1419x over previous
"""MoE linear (modality-routed) Trainium2 kernel.

out[n] = x[n] @ W[modality_ids[n]].T + b[modality_ids[n]]

Strategy (data parallel over 8 cores, weight replicated):
- Host: per-core shard of 16384 tokens, stable-sort tokens by expert and pad
  each expert segment to a shared 128-aligned capacity (one SPMD NEFF serves
  all cores; the expert of each 128-token subtile is a compile-time
  constant). x is cast to bf16 (rel tolerance 2e-2; bf16 matmul w/ fp32
  accum lands at ~4e-3) and pre-transposed into a [128, n_subtiles, 512]
  layout so every device DMA is a plain contiguous HWDGE transfer.
- Device per group of up to 4 subtiles (512 tokens): one ~512KB load of x^T,
  4 accumulating bf16 matmuls per subtile (lhsT = x^T chunk stationary,
  rhs = SBUF-resident W^T, fp32 PSUM), DVE bias-add + bf16 downcast, one
  ~512KB store of the y group.
- Host: invert the layout + permutation, upcast to fp32.
"""

import sys

if "/opt/trn_rl_repo" not in sys.path:
    sys.path.insert(0, "/opt/trn_rl_repo")

import ml_dtypes
import numpy as np

import concourse.bass as bass  # noqa: F401
import concourse.tile as tile
from concourse import bacc, mybir
from concourse.bass_utils import run_bass_kernel_spmd

N_CORES = 8
N_TOKENS = 131072
N_SHARD = N_TOKENS // N_CORES  # 16384
D_IN = 512
D_OUT = 512
N_EXPERTS = 3
P = 128
KC = D_IN // P  # 4 contraction chunks
GSUB = 4  # subtiles per group (DMA batch): 512 tokens

BF16 = ml_dtypes.bfloat16

_NC_CACHE = {}


def _groups_of(caps):
    """[(subtile_start, n_subtiles, expert), ...] with n_subtiles <= GSUB."""
    groups = []
    st = 0
    for e, c in enumerate(caps):
        n = c // P
        while n > 0:
            m = min(n, GSUB)
            groups.append((st, m, e))
            st += m
            n -= m
    return groups


def build_nc(caps, num_devices=N_CORES):
    """Build + compile the SPMD Bass kernel for given per-expert capacities
    (each a multiple of P)."""
    key = (tuple(caps), num_devices)
    if key in _NC_CACHE:
        return _NC_CACHE[key]
    npad = sum(caps)
    nst = npad // P
    groups = _groups_of(caps)

    nc = bacc.Bacc(
        "TRN2", target_bir_lowering=False, debug=False, num_devices=num_devices
    )
    f32 = mybir.dt.float32
    bf16 = mybir.dt.bfloat16

    # x^T, sorted by expert: xt[p, st, kc*P + t] = x_sorted[st*P + t, kc*P + p]
    xt = nc.dram_tensor("xt", [P, nst, KC * P], bf16, kind="ExternalInput").ap()
    # W^T blocks: wt[p, (e*KC+kc)*D_OUT + o] = W[e*D_OUT + o, kc*P + p]
    wt = nc.dram_tensor("wt", [P, N_EXPERTS * KC * D_OUT], bf16, kind="ExternalInput").ap()
    # bias broadcast across partitions: bb[p, e*D_OUT + o] = b[e*D_OUT + o]
    bb = nc.dram_tensor("bias_bc", [P, N_EXPERTS * D_OUT], f32, kind="ExternalInput").ap()
    # y[p, st, o] = y_sorted[st*P + p, o]
    y = nc.dram_tensor("y", [P, nst, D_OUT], bf16, kind="ExternalOutput").ap()

    with tile.TileContext(nc) as tc:
        with (
            tc.tile_pool(name="const", bufs=1) as cpool,
            tc.tile_pool(name="xg", bufs=8) as xg_pool,
            tc.tile_pool(name="outp", bufs=6) as out_pool,
            tc.tile_pool(name="pmm", bufs=8, space="PSUM") as pmm_pool,
        ):
            w_sb = cpool.tile([P, N_EXPERTS * KC * D_OUT], bf16)
            nc.sync.dma_start(out=w_sb[:], in_=wt[:])
            bias_sb = cpool.tile([P, N_EXPERTS * D_OUT], f32)
            nc.sync.dma_start(out=bias_sb[:], in_=bb[:])

            for st0, m, e in groups:
                xg = xg_pool.tile([P, m * KC * P], bf16)
                nc.sync.dma_start(out=xg[:], in_=xt[:, st0 : st0 + m, :])
                osb = out_pool.tile([P, m * D_OUT], bf16)
                for sub in range(m):
                    pmm = pmm_pool.tile([P, D_OUT], f32)
                    for kc in range(KC):
                        nc.tensor.matmul(
                            pmm[:],
                            lhsT=xg[
                                :, sub * KC * P + kc * P : sub * KC * P + (kc + 1) * P
                            ],
                            rhs=w_sb[
                                :, (e * KC + kc) * D_OUT : (e * KC + kc + 1) * D_OUT
                            ],
                            start=(kc == 0),
                            stop=(kc == KC - 1),
                        )
                    nc.vector.tensor_add(
                        out=osb[:, sub * D_OUT : (sub + 1) * D_OUT],
                        in0=pmm[:],
                        in1=bias_sb[:, e * D_OUT : (e + 1) * D_OUT],
                    )
                nc.sync.dma_start(out=y[:, st0 : st0 + m, :], in_=osb[:])

    nc.compile()
    _NC_CACHE[key] = nc
    return nc


def prepare(inputs):
    """Host-side prep: returns (nc, in_maps, posts) where posts[c] is
    (order, seg) needed to unscramble core c's output."""
    x = np.asarray(inputs["x"], dtype=np.float32)
    ids = np.asarray(inputs["modality_ids"]).astype(np.int64)
    weight = np.asarray(inputs["weight"], dtype=np.float32)
    b = np.asarray(inputs["bias"], dtype=np.float32)

    # W^T blocks in bf16: wt_dev[p, (e*KC+kc)*D_OUT + o] = W[e*D_OUT+o, kc*P+p]
    w3 = weight.reshape(N_EXPERTS, D_OUT, KC, P)  # [e, o, kc, p]
    wt_dev = np.ascontiguousarray(
        w3.transpose(3, 0, 2, 1).reshape(P, N_EXPERTS * KC * D_OUT)
    ).astype(BF16)
    bias_bc = np.ascontiguousarray(
        np.broadcast_to(b[None, :], (P, N_EXPERTS * D_OUT))
    ).astype(np.float32)

    counts = np.stack(
        [
            np.bincount(ids[c * N_SHARD : (c + 1) * N_SHARD], minlength=N_EXPERTS)
            for c in range(N_CORES)
        ]
    )
    caps = [int(-(-counts[:, e].max() // P) * P) for e in range(N_EXPERTS)]
    npad = sum(caps)
    nst = npad // P

    nc = build_nc(caps)
    in_maps = []
    posts = []
    xb = x.astype(BF16)
    for c in range(N_CORES):
        ids_c = ids[c * N_SHARD : (c + 1) * N_SHARD]
        order = np.argsort(ids_c, kind="stable").astype(np.int64)
        cnt = np.bincount(ids_c, minlength=N_EXPERTS)
        xs = np.zeros((npad, D_IN), dtype=BF16)
        base = 0
        off = 0
        seg = []  # (base, count) per expert, in sorted-order coords
        for e in range(N_EXPERTS):
            cc = int(cnt[e])
            xs[base : base + cc] = xb[c * N_SHARD : (c + 1) * N_SHARD][
                order[off : off + cc]
            ]
            seg.append((base, cc))
            base += caps[e]
            off += cc
        # xt_dev[p, st, kc*P + t] = xs[st*P + t, kc*P + p]
        xt_dev = np.ascontiguousarray(
            xs.reshape(nst, P, KC, P).transpose(3, 0, 2, 1).reshape(P, nst, KC * P)
        )
        in_maps.append({"xt": xt_dev, "wt": wt_dev, "bias_bc": bias_bc})
        posts.append((order, seg))
    return nc, in_maps, posts


def run(inputs, trace=False):
    """Returns (out, BassKernelResults)."""
    nc, in_maps, posts = prepare(inputs)
    res = run_bass_kernel_spmd(nc, in_maps, list(range(N_CORES)), trace=trace)
    out = np.empty((N_TOKENS, D_OUT), dtype=np.float32)
    for c in range(N_CORES):
        y_dev = np.asarray(res.results[c]["y"])  # [P, nst, D_OUT] bf16
        nst = y_dev.shape[1]
        # y_sorted[st*P + p, o] = y_dev[p, st, o]
        y_sorted = (
            y_dev.transpose(1, 0, 2).reshape(nst * P, D_OUT).astype(np.float32)
        )
        order, seg = posts[c]
        out_c = out[c * N_SHARD : (c + 1) * N_SHARD]
        off = 0
        for e in range(N_EXPERTS):
            base, cc = seg[e]
            out_c[order[off : off + cc]] = y_sorted[base : base + cc]
            off += cc
    return out, res


def kernel(**inputs):
    out, _ = run(inputs, trace=False)
    return out


# revision 24
# speedup vs baseline: 111.9957x; 1.1649x over previous
"""MoE linear (modality-routed) Trainium2 kernel.

out[n] = x[n] @ W[modality_ids[n]].T + b[modality_ids[n]]

Strategy (data parallel over 8 cores, weight replicated):
- Host: per-core shard of 16384 tokens, stable-sort tokens by expert and pad
  each expert segment to a shared 128-aligned capacity (one SPMD NEFF serves
  all cores; the expert of each 128-token subtile is a compile-time
  constant). x is cast to bf16 (rel tolerance 2e-2; bf16 matmul w/ fp32
  accum lands at ~4e-3) and pre-transposed into a [128, n_subtiles, 512]
  layout so every device DMA is a plain contiguous HWDGE transfer.
- Device per group of up to 4 subtiles (512 tokens): one ~512KB load of x^T,
  4 accumulating bf16 matmuls per subtile (lhsT = x^T chunk stationary,
  rhs = SBUF-resident W^T, fp32 PSUM), DVE bias-add + bf16 downcast, one
  ~512KB store of the y group.
- Host: invert the layout + permutation, upcast to fp32.
"""

import sys

if "/opt/trn_rl_repo" not in sys.path:
    sys.path.insert(0, "/opt/trn_rl_repo")

import ml_dtypes
import numpy as np

import concourse.bass as bass  # noqa: F401
import concourse.tile as tile
from concourse import bacc, mybir
from concourse.bass_utils import run_bass_kernel_spmd

N_CORES = 8
N_TOKENS = 131072
N_SHARD = N_TOKENS // N_CORES  # 16384
D_IN = 512
D_OUT = 512
N_EXPERTS = 3
P = 128
KC = D_IN // P  # 4 contraction chunks
GSUB = 4  # subtiles per group (DMA batch): 512 tokens

BF16 = ml_dtypes.bfloat16

_NC_CACHE = {}


def _groups_of(caps):
    """[(subtile_start, n_subtiles, expert), ...] with n_subtiles <= GSUB."""
    groups = []
    st = 0
    for e, c in enumerate(caps):
        n = c // P
        while n > 0:
            m = min(n, GSUB)
            groups.append((st, m, e))
            st += m
            n -= m
    return groups


def build_nc(caps, num_devices=N_CORES):
    """Build + compile the SPMD Bass kernel for given per-expert capacities
    (each a multiple of P)."""
    key = (tuple(caps), num_devices)
    if key in _NC_CACHE:
        return _NC_CACHE[key]
    npad = sum(caps)
    nst = npad // P
    groups = _groups_of(caps)

    nc = bacc.Bacc(
        "TRN2", target_bir_lowering=False, debug=False, num_devices=num_devices
    )
    f32 = mybir.dt.float32
    bf16 = mybir.dt.bfloat16

    # x^T, sorted by expert: xt[p, st, kc*P + t] = x_sorted[st*P + t, kc*P + p]
    xt = nc.dram_tensor("xt", [P, nst, KC * P], bf16, kind="ExternalInput").ap()
    # W^T blocks: wt[p, (e*KC+kc)*D_OUT + o] = W[e*D_OUT + o, kc*P + p]
    wt = nc.dram_tensor("wt", [P, N_EXPERTS * KC * D_OUT], bf16, kind="ExternalInput").ap()
    # bias broadcast across partitions: bb[p, e*D_OUT + o] = b[e*D_OUT + o]
    bb = nc.dram_tensor("bias_bc", [P, N_EXPERTS * D_OUT], bf16, kind="ExternalInput").ap()
    # y[p, st, o] = y_sorted[st*P + p, o]
    y = nc.dram_tensor("y", [P, nst, D_OUT], bf16, kind="ExternalOutput").ap()

    with tile.TileContext(nc) as tc:
        with (
            tc.tile_pool(name="const", bufs=1) as cpool,
            tc.tile_pool(name="xg", bufs=8) as xg_pool,
            tc.tile_pool(name="outp", bufs=6) as out_pool,
            tc.tile_pool(name="pmm", bufs=7, space="PSUM") as pmm_pool,
            tc.tile_pool(name="wps", bufs=1, space="PSUM") as wps_pool,
        ):
            w_sb = cpool.tile([P, N_EXPERTS * KC * D_OUT], bf16)
            bias_sb = cpool.tile([P, N_EXPERTS * D_OUT], bf16)

            # PE warmup: tiny matmuls fill the otherwise-idle DMA startup
            # window and get the PE clock ramp to full speed before the first
            # real matmul is issued. They read a w_sb slice whose DMA arrives
            # much later (expert 2's last chunk), so the only dependency is a
            # harmless write-after-read on that late DMA.
            warm = w_sb[:, N_EXPERTS * KC * D_OUT - 64 :]
            wps = wps_pool.tile([64, 64], f32)
            for _ in range(94):
                nc.tensor.matmul(
                    wps[:], lhsT=warm, rhs=warm, start=True, stop=True
                )

            ngroups = len(groups)
            # Experts 1/2 weights+bias stream in as small per-kc pieces spread
            # over the expert-0/1 phases, amortized into per-group DMA slack.
            ng_e = [(caps[e] // P + GSUB - 1) // GSUB for e in range(N_EXPERTS)]
            const_sched = {}  # gi -> list of (w_sb slice, wt slice) column ranges
            for e in (1, 2):
                if caps[e] == 0:
                    continue
                need_by = sum(ng_e[:e])  # first group index of expert e
                start = max(1, need_by - 7)
                pieces = [
                    ((e * KC + kc) * D_OUT, (e * KC + kc + 1) * D_OUT)
                    for kc in range(KC)
                ]
                for i, piece in enumerate(pieces):
                    const_sched.setdefault(min(start + i, need_by - 1), []).append(
                        ("w", piece)
                    )
                const_sched.setdefault(min(start + KC, need_by - 1), []).append(
                    ("b", (e * D_OUT, (e + 1) * D_OUT))
                )
            for gi, (st0, m, e) in enumerate(groups):
                xg = xg_pool.tile([P, m * KC * P], bf16)
                nc.sync.dma_start(out=xg[:], in_=xt[:, st0 : st0 + m, :])
                if gi == 0:
                    # First group's weights arrive per-kc chunk right behind
                    # its x tile; bias + experts 1/2 stream in later, hidden
                    # behind compute.
                    for kc in range(KC):
                        nc.sync.dma_start(
                            out=w_sb[:, kc * D_OUT : (kc + 1) * D_OUT],
                            in_=wt[:, kc * D_OUT : (kc + 1) * D_OUT],
                        )
                    nc.sync.dma_start(
                        out=bias_sb[:, :D_OUT], in_=bb[:, :D_OUT]
                    )
                osb = out_pool.tile([P, m * D_OUT], bf16)
                last_group = gi == ngroups - 1
                for sub in range(m):
                    pmm = pmm_pool.tile([P, D_OUT], f32)
                    for kc in range(KC):
                        nc.tensor.matmul(
                            pmm[:],
                            lhsT=xg[
                                :, sub * KC * P + kc * P : sub * KC * P + (kc + 1) * P
                            ],
                            rhs=w_sb[
                                :, (e * KC + kc) * D_OUT : (e * KC + kc + 1) * D_OUT
                            ],
                            start=(kc == 0),
                            stop=(kc == KC - 1),
                        )
                    nc.vector.tensor_add(
                        out=osb[:, sub * D_OUT : (sub + 1) * D_OUT],
                        in0=pmm[:],
                        in1=bias_sb[:, e * D_OUT : (e + 1) * D_OUT],
                    )
                    if last_group:
                        # Per-subtile stores so earlier stores overlap the
                        # remaining matmuls and the final transfer is small.
                        nc.sync.dma_start(
                            out=y[:, st0 + sub, :],
                            in_=osb[:, sub * D_OUT : (sub + 1) * D_OUT],
                        )
                if not last_group:
                    nc.sync.dma_start(out=y[:, st0 : st0 + m, :], in_=osb[:])
                for kind, (lo, hi) in const_sched.get(gi, ()):
                    if kind == "w":
                        nc.sync.dma_start(out=w_sb[:, lo:hi], in_=wt[:, lo:hi])
                    else:
                        nc.sync.dma_start(out=bias_sb[:, lo:hi], in_=bb[:, lo:hi])

    nc.compile()
    _NC_CACHE[key] = nc
    return nc


def prepare(inputs):
    """Host-side prep: returns (nc, in_maps, posts) where posts[c] is
    (order, seg) needed to unscramble core c's output."""
    x = np.asarray(inputs["x"], dtype=np.float32)
    ids = np.asarray(inputs["modality_ids"]).astype(np.int64)
    weight = np.asarray(inputs["weight"], dtype=np.float32)
    b = np.asarray(inputs["bias"], dtype=np.float32)

    # W^T blocks in bf16: wt_dev[p, (e*KC+kc)*D_OUT + o] = W[e*D_OUT+o, kc*P+p]
    w3 = weight.reshape(N_EXPERTS, D_OUT, KC, P)  # [e, o, kc, p]
    wt_dev = np.ascontiguousarray(
        w3.transpose(3, 0, 2, 1).reshape(P, N_EXPERTS * KC * D_OUT)
    ).astype(BF16)
    bias_bc = np.ascontiguousarray(
        np.broadcast_to(b[None, :], (P, N_EXPERTS * D_OUT))
    ).astype(BF16)

    # Balanced sharding: tokens of each expert are dealt near-evenly across the
    # 8 cores (the shard assignment is ours to choose — we un-permute at the
    # end), which minimizes the shared per-expert capacity padding.
    chunks = []  # chunks[e][c] = global token indices of expert e on core c
    for e in range(N_EXPERTS):
        idx_e = np.nonzero(ids == e)[0]
        chunks.append(np.array_split(idx_e, N_CORES))
    caps = [
        int(-(-max(len(ch) for ch in chunks[e]) // P) * P)
        for e in range(N_EXPERTS)
    ]
    npad = sum(caps)
    nst = npad // P

    nc = build_nc(caps)
    in_maps = []
    posts = []
    xb = x.astype(BF16)
    for c in range(N_CORES):
        xs = np.zeros((npad, D_IN), dtype=BF16)
        base = 0
        seg = []  # (global_indices, base) per expert
        for e in range(N_EXPERTS):
            gidx = chunks[e][c]
            cc = len(gidx)
            xs[base : base + cc] = xb[gidx]
            seg.append((gidx, base))
            base += caps[e]
        # xt_dev[p, st, kc*P + t] = xs[st*P + t, kc*P + p]
        xt_dev = np.ascontiguousarray(
            xs.reshape(nst, P, KC, P).transpose(3, 0, 2, 1).reshape(P, nst, KC * P)
        )
        in_maps.append({"xt": xt_dev, "wt": wt_dev, "bias_bc": bias_bc})
        posts.append(seg)
    return nc, in_maps, posts


def run(inputs, trace=False):
    """Returns (out, BassKernelResults)."""
    nc, in_maps, posts = prepare(inputs)
    res = run_bass_kernel_spmd(nc, in_maps, list(range(N_CORES)), trace=trace)
    out = np.empty((N_TOKENS, D_OUT), dtype=np.float32)
    for c in range(N_CORES):
        y_dev = np.asarray(res.results[c]["y"])  # [P, nst, D_OUT] bf16
        nst = y_dev.shape[1]
        # y_sorted[st*P + p, o] = y_dev[p, st, o]
        y_sorted = (
            y_dev.transpose(1, 0, 2).reshape(nst * P, D_OUT).astype(np.float32)
        )
        for gidx, base in posts[c]:
            out[gidx] = y_sorted[base : base + len(gidx)]
    return out, res


def kernel(**inputs):
    out, _ = run(inputs, trace=False)
    return out


# revision 28
# speedup vs baseline: 112.0042x; 1.0001x over previous
"""MoE linear (modality-routed) Trainium2 kernel.

out[n] = x[n] @ W[modality_ids[n]].T + b[modality_ids[n]]

Strategy (data parallel over 8 cores, weight replicated):
- Host: per-core shard of 16384 tokens, stable-sort tokens by expert and pad
  each expert segment to a shared 128-aligned capacity (one SPMD NEFF serves
  all cores; the expert of each 128-token subtile is a compile-time
  constant). x is cast to bf16 (rel tolerance 2e-2; bf16 matmul w/ fp32
  accum lands at ~4e-3) and pre-transposed into a [128, n_subtiles, 512]
  layout so every device DMA is a plain contiguous HWDGE transfer.
- Device per group of up to 4 subtiles (512 tokens): one ~512KB load of x^T,
  4 accumulating bf16 matmuls per subtile (lhsT = x^T chunk stationary,
  rhs = SBUF-resident W^T, fp32 PSUM), DVE bias-add + bf16 downcast, one
  ~512KB store of the y group.
- Host: invert the layout + permutation, upcast to fp32.
"""

import sys

if "/opt/trn_rl_repo" not in sys.path:
    sys.path.insert(0, "/opt/trn_rl_repo")

import ml_dtypes
import numpy as np

import concourse.bass as bass  # noqa: F401
import concourse.tile as tile
from concourse import bacc, mybir
from concourse.bass_utils import run_bass_kernel_spmd

N_CORES = 8
N_TOKENS = 131072
N_SHARD = N_TOKENS // N_CORES  # 16384
D_IN = 512
D_OUT = 512
N_EXPERTS = 3
P = 128
KC = D_IN // P  # 4 contraction chunks
GSUB = 4  # subtiles per group (DMA batch): 512 tokens

BF16 = ml_dtypes.bfloat16
WARMUPS = 140  # PE clock-ramp warmup matmuls

_NC_CACHE = {}


def _groups_of(caps):
    """[(subtile_start, n_subtiles, expert), ...] with n_subtiles <= GSUB."""
    groups = []
    st = 0
    for e, c in enumerate(caps):
        n = c // P
        while n > 0:
            m = min(n, GSUB)
            groups.append((st, m, e))
            st += m
            n -= m
    return groups


def build_nc(caps, num_devices=N_CORES):
    """Build + compile the SPMD Bass kernel for given per-expert capacities
    (each a multiple of P)."""
    key = (tuple(caps), num_devices)
    if key in _NC_CACHE:
        return _NC_CACHE[key]
    npad = sum(caps)
    nst = npad // P
    groups = _groups_of(caps)

    nc = bacc.Bacc(
        "TRN2", target_bir_lowering=False, debug=False, num_devices=num_devices
    )
    f32 = mybir.dt.float32
    bf16 = mybir.dt.bfloat16

    # x^T, sorted by expert: xt[p, st, kc*P + t] = x_sorted[st*P + t, kc*P + p]
    xt = nc.dram_tensor("xt", [P, nst, KC * P], bf16, kind="ExternalInput").ap()
    # W^T blocks: wt[p, (e*KC+kc)*D_OUT + o] = W[e*D_OUT + o, kc*P + p]
    wt = nc.dram_tensor("wt", [P, N_EXPERTS * KC * D_OUT], bf16, kind="ExternalInput").ap()
    # bias broadcast across partitions: bb[p, e*D_OUT + o] = b[e*D_OUT + o]
    bb = nc.dram_tensor("bias_bc", [P, N_EXPERTS * D_OUT], bf16, kind="ExternalInput").ap()
    # y[p, st, o] = y_sorted[st*P + p, o]
    y = nc.dram_tensor("y", [P, nst, D_OUT], bf16, kind="ExternalOutput").ap()

    with tile.TileContext(nc) as tc:
        with (
            tc.tile_pool(name="const", bufs=1) as cpool,
            tc.tile_pool(name="xg", bufs=8) as xg_pool,
            tc.tile_pool(name="outp", bufs=6) as out_pool,
            tc.tile_pool(name="pmm", bufs=7, space="PSUM") as pmm_pool,
            tc.tile_pool(name="wps", bufs=1, space="PSUM") as wps_pool,
        ):
            w_sb = cpool.tile([P, N_EXPERTS * KC * D_OUT], bf16)
            bias_sb = cpool.tile([P, N_EXPERTS * D_OUT], bf16)

            # PE warmup: tiny matmuls fill the otherwise-idle DMA startup
            # window and get the PE clock ramp to full speed before the first
            # real matmul is issued. They read a w_sb slice whose DMA arrives
            # much later (expert 2's last chunk), so the only dependency is a
            # harmless write-after-read on that late DMA.
            warm = w_sb[:, N_EXPERTS * KC * D_OUT - 32 :]
            wps = wps_pool.tile([32, 32], f32)
            for _ in range(WARMUPS):
                nc.tensor.matmul(
                    wps[:], lhsT=warm, rhs=warm, start=True, stop=True
                )

            ngroups = len(groups)
            e_first = groups[0][2]
            # Later experts' weights+bias stream in as small per-kc pieces
            # spread over earlier phases, amortized into per-group DMA slack.
            first_gi_of_e = {}
            for gi, (_, _, e) in enumerate(groups):
                first_gi_of_e.setdefault(e, gi)
            const_sched = {}  # gi -> [("w"|"b", (lo, hi) column range)]
            for e in range(N_EXPERTS):
                if caps[e] == 0 or e == e_first:
                    continue
                need_by = max(1, first_gi_of_e[e])
                start = max(1, need_by - 7)
                for kc in range(KC):
                    const_sched.setdefault(min(start + kc, need_by - 1), []).append(
                        ("w", ((e * KC + kc) * D_OUT, (e * KC + kc + 1) * D_OUT))
                    )
                const_sched.setdefault(min(start + KC, need_by - 1), []).append(
                    ("b", (e * D_OUT, (e + 1) * D_OUT))
                )
            for gi, (st0, m, e) in enumerate(groups):
                xg = xg_pool.tile([P, m * KC * P], bf16)
                nc.sync.dma_start(out=xg[:], in_=xt[:, st0 : st0 + m, :])
                if gi == 0:
                    # First group's weights arrive per-kc chunk right behind
                    # its x tile; bias + experts 1/2 stream in later, hidden
                    # behind compute.
                    for kc in range(KC):
                        nc.sync.dma_start(
                            out=w_sb[:, kc * D_OUT : (kc + 1) * D_OUT],
                            in_=wt[:, kc * D_OUT : (kc + 1) * D_OUT],
                        )
                    nc.sync.dma_start(
                        out=bias_sb[:, :D_OUT], in_=bb[:, :D_OUT]
                    )
                osb = out_pool.tile([P, m * D_OUT], bf16)
                last_group = gi == ngroups - 1
                for sub in range(m):
                    pmm = pmm_pool.tile([P, D_OUT], f32)
                    for kc in range(KC):
                        nc.tensor.matmul(
                            pmm[:],
                            lhsT=xg[
                                :, sub * KC * P + kc * P : sub * KC * P + (kc + 1) * P
                            ],
                            rhs=w_sb[
                                :, (e * KC + kc) * D_OUT : (e * KC + kc + 1) * D_OUT
                            ],
                            start=(kc == 0),
                            stop=(kc == KC - 1),
                        )
                    nc.vector.tensor_add(
                        out=osb[:, sub * D_OUT : (sub + 1) * D_OUT],
                        in0=pmm[:],
                        in1=bias_sb[:, e * D_OUT : (e + 1) * D_OUT],
                    )
                    if last_group:
                        # Per-subtile stores so earlier stores overlap the
                        # remaining matmuls and the final transfer is small.
                        nc.sync.dma_start(
                            out=y[:, st0 + sub, :],
                            in_=osb[:, sub * D_OUT : (sub + 1) * D_OUT],
                        )
                if not last_group:
                    nc.sync.dma_start(out=y[:, st0 : st0 + m, :], in_=osb[:])
                for kind, (lo, hi) in const_sched.get(gi, ()):
                    if kind == "w":
                        nc.sync.dma_start(out=w_sb[:, lo:hi], in_=wt[:, lo:hi])
                    else:
                        nc.sync.dma_start(out=bias_sb[:, lo:hi], in_=bb[:, lo:hi])

    nc.compile()
    _NC_CACHE[key] = nc
    return nc


def prepare(inputs):
    """Host-side prep: returns (nc, in_maps, posts) where posts[c] is
    (order, seg) needed to unscramble core c's output."""
    x = np.asarray(inputs["x"], dtype=np.float32)
    ids = np.asarray(inputs["modality_ids"]).astype(np.int64)
    weight = np.asarray(inputs["weight"], dtype=np.float32)
    b = np.asarray(inputs["bias"], dtype=np.float32)

    # W^T blocks in bf16: wt_dev[p, (e*KC+kc)*D_OUT + o] = W[e*D_OUT+o, kc*P+p]
    w3 = weight.reshape(N_EXPERTS, D_OUT, KC, P)  # [e, o, kc, p]
    wt_dev = np.ascontiguousarray(
        w3.transpose(3, 0, 2, 1).reshape(P, N_EXPERTS * KC * D_OUT)
    ).astype(BF16)
    bias_bc = np.ascontiguousarray(
        np.broadcast_to(b[None, :], (P, N_EXPERTS * D_OUT))
    ).astype(BF16)

    # Balanced sharding: tokens of each expert are dealt near-evenly across the
    # 8 cores (the shard assignment is ours to choose — we un-permute at the
    # end), which minimizes the shared per-expert capacity padding.
    chunks = []  # chunks[e][c] = global token indices of expert e on core c
    for e in range(N_EXPERTS):
        idx_e = np.nonzero(ids == e)[0]
        chunks.append(np.array_split(idx_e, N_CORES))
    caps = [
        int(-(-max(len(ch) for ch in chunks[e]) // P) * P)
        for e in range(N_EXPERTS)
    ]
    npad = sum(caps)
    nst = npad // P

    nc = build_nc(caps)
    in_maps = []
    posts = []
    xb = x.astype(BF16)
    for c in range(N_CORES):
        xs = np.zeros((npad, D_IN), dtype=BF16)
        base = 0
        seg = []  # (global_indices, base) per expert
        for e in range(N_EXPERTS):
            gidx = chunks[e][c]
            cc = len(gidx)
            xs[base : base + cc] = xb[gidx]
            seg.append((gidx, base))
            base += caps[e]
        # xt_dev[p, st, kc*P + t] = xs[st*P + t, kc*P + p]
        xt_dev = np.ascontiguousarray(
            xs.reshape(nst, P, KC, P).transpose(3, 0, 2, 1).reshape(P, nst, KC * P)
        )
        in_maps.append({"xt": xt_dev, "wt": wt_dev, "bias_bc": bias_bc})
        posts.append(seg)
    return nc, in_maps, posts


def run(inputs, trace=False):
    """Returns (out, BassKernelResults)."""
    nc, in_maps, posts = prepare(inputs)
    res = run_bass_kernel_spmd(nc, in_maps, list(range(N_CORES)), trace=trace)
    out = np.empty((N_TOKENS, D_OUT), dtype=np.float32)
    for c in range(N_CORES):
        y_dev = np.asarray(res.results[c]["y"])  # [P, nst, D_OUT] bf16
        nst = y_dev.shape[1]
        # y_sorted[st*P + p, o] = y_dev[p, st, o]
        y_sorted = (
            y_dev.transpose(1, 0, 2).reshape(nst * P, D_OUT).astype(np.float32)
        )
        for gidx, base in posts[c]:
            out[gidx] = y_sorted[base : base + len(gidx)]
    return out, res


def kernel(**inputs):
    out, _ = run(inputs, trace=False)
    return out
